# revision 55
# baseline (speedup 1.0000x reference)
import os
import sys

import numpy as np

for _p in ("/opt/trn_rl_repo", os.path.expanduser("~/.axon_site/_ro/trn_rl_repo")):
    if os.path.isdir(_p) and _p not in sys.path:
        sys.path.insert(0, _p)

import ml_dtypes

import concourse.bacc as bacc
import concourse.mybir as mybir
import concourse.tile as tile
from concourse import bass_utils
from concourse.library_config import mlp as mlp_lib


F32 = mybir.dt.float32
BF16 = mybir.dt.bfloat16
I16 = mybir.dt.int16
AF = mybir.ActivationFunctionType
ALU = mybir.AluOpType
AX = mybir.AxisListType

NUM_USER = 200000
NUM_ITEM = 200000
DIM_E = 64
DIM_FEAT = 128
B = 16384
G = 17
TEMP = 0.2
LR_LAMBDA = 0.5

NCORE = 8
BC = B // NCORE
NT = BC // 128
NBP = NT // 4
EPT = 128 * G
ICOL = EPT // 16
CH = 512

GCALL = int(os.environ.get("KERNEL_GCALL", "1024"))
assert GCALL % 128 == 0
CPS = -(-4 * EPT // GCALL)

_CACHE: dict = {}

_NEEDED_AF = None


def _patch_act_tables():
    global _NEEDED_AF
    if _CACHE.get("act_patched"):
        return
    _NEEDED_AF = {AF.Ln, AF.Exp, AF.Prelu, AF.Copy, AF.Identity}
    import concourse.hw_specs as hw_specs
    orig = hw_specs.get_activation_tables

    def patched(module_arch):
        tabs = orig(module_arch)
        out = {}
        for name, fns in tabs.items():
            if name == "natural_log_exp_and_others":
                out[name] = fns
            else:
                out[name] = fns - _NEEDED_AF
        return out

    bacc.get_activation_tables = patched
    _CACHE["act_patched"] = True


def _build(NS4, S_call):
    _patch_act_tables()
    NCH = NS4 // CH
    GRP = int(os.environ.get('KERNEL_GRP', '4'))
    GW = GRP * CH

    nc = bacc.Bacc("TRN2", target_bir_lowering=False, debug=False,
                   num_devices=NCORE, num_swdge_queues=4)

    vet_d = nc.dram_tensor("vet", [128, NS4], BF16, kind="ExternalInput")
    w1_d = nc.dram_tensor("w1", [DIM_FEAT, 256], BF16, kind="ExternalInput")
    b1_d = nc.dram_tensor("b1", [256], F32, kind="ExternalInput")
    w2_d = nc.dram_tensor("w2", [128, 128], BF16, kind="ExternalInput")
    b2r_d = nc.dram_tensor("b2r", [128, DIM_E], BF16, kind="ExternalInput")
    i2i_d = nc.dram_tensor("i2i", [128, NT * ICOL], I16, kind="ExternalInput")
    ue_d = nc.dram_tensor("ue", [128, NT * G * 64], BF16, kind="ExternalInput")
    qa_d = nc.dram_tensor("qa", [128, NT * G], F32, kind="ExternalInput")
    pe_d = nc.dram_tensor("peb", [128, NT * DIM_E], F32, kind="ExternalInput")
    mk_d = nc.dram_tensor("mask", [128, NT * G], F32, kind="ExternalInput")
    out_d = nc.dram_tensor("acc_out", [128, 4 * NT], F32, kind="ExternalOutput")

    scri_d = nc.dram_tensor("scri", [NS4, 128], BF16, kind="Internal")

    with tile.TileContext(nc) as tc:
        nc.gpsimd.load_library(mlp_lib)
        with tc.tile_pool(name="const", bufs=1) as cp:
            w1sb = cp.tile([128, 256], BF16, tag="w1sb")
            nc.sync.dma_start(out=w1sb[:], in_=w1_d[:])
            w2sb = cp.tile([128, 128], BF16, tag="w2sb")
            nc.sync.dma_start(out=w2sb[:], in_=w2_d[:])
            b1sb = cp.tile([128, 2], F32, tag="b1sb")
            nc.sync.dma_start(out=b1sb[:], in_=b1_d[:].rearrange("(h p) -> p h", p=128))
            b2r = cp.tile([128, DIM_E], BF16, tag="b2r")
            nc.sync.dma_start(out=b2r[:], in_=b2r_d[:])
            acc = cp.tile([128, 4 * NT], F32, tag="acc")

            qrr = [0]

            def gather_split(dst3, src, idxs, coff0, total):
                done = 0
                while done < total:
                    n = min(GCALL, total - done)
                    nc.gpsimd.dma_gather(
                        dst3[:, done // 128:(done + n) // 128, :],
                        src,
                        idxs[:, coff0 + done // 16:coff0 + (done + n) // 16],
                        n, n, 128, queue_num=qrr[0] % 4)
                    qrr[0] += 1
                    done += n

            with tc.tile_pool(name="enc", bufs=3) as ep, \
                 tc.tile_pool(name="ph2", bufs=3) as pp, \
                 tc.tile_pool(name="prd", bufs=2) as prp, \
                 tc.tile_pool(name="psH", bufs=3, space="PSUM") as psH, \
                 tc.tile_pool(name="psO", bufs=3, space="PSUM") as psO:

                i2i = pp.tile([128, NT * ICOL], I16, tag="i2i", bufs=1)
                nc.sync.dma_start(out=i2i[:], in_=i2i_d[:])
                ssf = pp.tile([128, NS4 // 128], F32, tag="ssf", bufs=1)

                pe_all = pp.tile([128, NT * DIM_E], F32, tag="peA", bufs=1)
                nc.sync.dma_start(out=pe_all[:], in_=pe_d[:])
                mk_all = pp.tile([128, NT * G], F32, tag="mkA", bufs=1)
                nc.sync.dma_start(out=mk_all[:], in_=mk_d[:])
                qa_all = pp.tile([128, NT * G], F32, tag="qaA", bufs=1)
                nc.sync.dma_start(out=qa_all[:], in_=qa_d[:])
                prP = pp.tile([128, NT * DIM_E], F32, tag="prP", bufs=1)
                nc.vector.tensor_tensor(out=prP[:], in0=pe_all[:],
                                        in1=pe_all[:], op=ALU.mult)
                dPPa = pp.tile([128, NT], F32, tag="dPPa", bufs=1)
                nc.vector.tensor_reduce(
                    out=dPPa[:],
                    in_=prP[:].rearrange("p (t e) -> p t e", e=DIM_E),
                    op=ALU.add, axis=AX.X)
                nc.scalar.activation(out=dPPa[:], in_=dPPa[:], func=AF.Ln)
                nc.scalar.activation(out=dPPa[:], in_=dPPa[:], func=AF.Exp,
                                     scale=-0.5)
                phA = pp.tile([128, NT * DIM_E], BF16, tag="phA", bufs=1)
                nc.vector.tensor_tensor(
                    out=phA[:].rearrange("p (t e) -> p t e", e=DIM_E),
                    in0=pe_all[:].rearrange("p (t e) -> p t e", e=DIM_E),
                    in1=dPPa[:].rearrange("p (t o) -> p t o", o=1)
                        .to_broadcast([128, NT, DIM_E]),
                    op=ALU.mult)

                def enc_groupA(g0):
                    veT = ep.tile([128, GW], BF16, tag="veT", bufs=2)
                    nc.sync.dma_start(out=veT[:],
                                      in_=vet_d[:, g0 * CH:g0 * CH + GW])
                    hs = []
                    for c in range(GRP):
                        h_sb = ep.tile([128, 2 * CH], BF16, tag="hsb", bufs=10)
                        for h in range(2):
                            h_ps = psH.tile([128, CH], F32, tag="hp")
                            nc.tensor.matmul(
                                out=h_ps[:],
                                lhsT=w1sb[:, h * 128:(h + 1) * 128],
                                rhs=veT[:, c * CH:(c + 1) * CH],
                                start=True, stop=True)
                            nc.scalar.activation(
                                out=h_sb[:, h * CH:(h + 1) * CH], in_=h_ps[:],
                                func=AF.Prelu, bias=b1sb[:, h:h + 1],
                                scale=1.0, alpha=0.01)
                        hs.append(h_sb)
                    return hs

                def enc_groupB(g0, hs):
                    scG = ep.tile([128, GW], BF16, tag="scG", bufs=2)
                    scG3 = scG[:].rearrange("p (j e) -> p j e", e=128)
                    fPb = ep.tile([128, GW // 2], BF16, tag="fPb", bufs=2)
                    fPb3 = fPb[:].rearrange("p (j e) -> p j e", e=64)
                    for c in range(GRP):
                        h_sb = hs[c]
                        Fo_ps = psO.tile([128, CH // 2], F32, tag="fo")
                        for j in range(4):
                            nc.tensor.matmul(
                                out=Fo_ps[:, j * 64:(j + 1) * 64],
                                lhsT=h_sb[:, j * 128:(j + 1) * 128],
                                rhs=w2sb[:, 0:64], start=True, stop=False)
                            nc.tensor.matmul(
                                out=Fo_ps[:, j * 64:(j + 1) * 64],
                                lhsT=h_sb[:, CH + j * 128:CH + (j + 1) * 128],
                                rhs=w2sb[:, 64:128], start=False, stop=True)
                        nc.scalar.activation(
                            out=fPb[:, c * 256:(c + 1) * 256], in_=Fo_ps[:],
                            func=AF.Identity)
                    so = g0 * 4
                    fB = ep.tile([128, GW // 2], BF16, tag="fB", bufs=2)
                    fB3 = fB[:].rearrange("p (j e) -> p j e", e=64)
                    nc.vector.tensor_tensor(
                        out=fB3, in0=fPb3,
                        in1=b2r[:].rearrange("p (j e) -> p j e", j=1)
                            .to_broadcast([128, 4 * GRP, 64]),
                        op=ALU.add)
                    sq = ep.tile([128, GW // 2], BF16, tag="sq", bufs=2)
                    sq3 = sq[:].rearrange("p (j e) -> p j e", e=64)
                    nc.vector.tensor_tensor(out=sq3, in0=fB3, in1=fB3,
                                            op=ALU.mult)
                    nc.vector.tensor_reduce(out=ssf[:, so:so + 4 * GRP],
                                            in_=sq3, op=ALU.add, axis=AX.X)
                    nc.scalar.activation(out=ssf[:, so:so + 4 * GRP],
                                         in_=ssf[:, so:so + 4 * GRP],
                                         func=AF.Ln)
                    nc.scalar.activation(out=ssf[:, so:so + 4 * GRP],
                                         in_=ssf[:, so:so + 4 * GRP],
                                         func=AF.Exp, scale=-0.5)
                    invb = ep.tile([128, 4 * GRP], BF16, tag="invb", bufs=2)
                    nc.vector.tensor_copy(out=invb[:], in_=ssf[:, so:so + 4 * GRP])
                    fhG = ep.tile([128, GW // 2], BF16, tag="fhG", bufs=2)
                    fhG3 = fhG[:].rearrange("p (j e) -> p j e", e=64)
                    nc.vector.tensor_tensor(
                        out=fhG3, in0=fB3,
                        in1=invb[:].rearrange("p (j o) -> p j o", o=1)
                            .to_broadcast([128, 4 * GRP, 64]),
                        op=ALU.mult)
                    nc.vector.tensor_copy(out=scG3[:, :, 0:64], in_=fhG3)
                    nc.vector.tensor_copy(out=scG3[:, :, 64:128], in_=fPb3)
                    nc.sync.dma_start(
                        out=scri_d[g0 * CH:g0 * CH + GW, :]
                            .rearrange("(j p) e -> p j e", p=128),
                        in_=scG)

                S_sup = [max(S_call[bp * CPS:(bp + 1) * CPS])
                         for bp in range(NBP)]

                def ph2_gather(bp):
                    ti_p = pp.tile([128, 4 * G * 128], BF16, tag="ti", bufs=3)
                    ti_p3 = ti_p[:].rearrange("p (g e) -> p g e", e=128)
                    gather_split(ti_p3, scri_d[0:S_sup[bp], :], i2i,
                                 4 * bp * ICOL, 4 * EPT)
                    return ti_p

                def ph2_load_ue(bp):
                    ue_p = pp.tile([128, 4 * G * 64], BF16, tag="ue", bufs=3)
                    nc.sync.dma_start(
                        out=ue_p[:],
                        in_=ue_d[:, 4 * bp * G * 64:(4 * bp + 4) * G * 64])
                    return ue_p

                def ph2_gather_tile(bp, t, ti_p):
                    lo = (4 * t * EPT) // (4 * GCALL)
                    hi = ((t + 1) * EPT - 1) // GCALL
                    pref = max(S_call[bp * CPS + k]
                               for k in range(lo, hi + 1))
                    ti_p3 = ti_p[:].rearrange("p (g e) -> p g e", e=128)
                    done = t * EPT
                    end = (t + 1) * EPT
                    while done < end:
                        n = min(GCALL, end - done)
                        nc.gpsimd.dma_gather(
                            ti_p3[:, done // 128:(done + n) // 128, :],
                            scri_d[0:pref, :],
                            i2i[:, 4 * bp * ICOL + done // 16:
                                4 * bp * ICOL + (done + n) // 16],
                            n, n, 128, queue_num=qrr[0] % 4)
                        qrr[0] += 1
                        done += n

                def ph2_compute(bp, t, ti_p, ue_p):
                    bt = 4 * bp + t
                    ti3 = ti_p[:, t * G * 128:(t + 1) * G * 128] \
                        .rearrange("p (g e) -> p g e", e=128)
                    ue3 = ue_p[:, t * G * 64:(t + 1) * G * 64] \
                        .rearrange("p (g e) -> p g e", e=64)
                    ph3b = phA[:, bt * DIM_E:(bt + 1) * DIM_E] \
                        .rearrange("p (g e) -> p g e", g=1) \
                        .to_broadcast([128, G, 64])
                    pr = prp.tile([128, G * 64], BF16, tag="pr")
                    pr3 = pr[:].rearrange("p (g e) -> p g e", e=64)
                    d1 = pp.tile([128, G], F32, tag="d1")
                    nc.vector.tensor_tensor(out=pr3, in0=ti3[:, :, 0:64],
                                            in1=ph3b, op=ALU.mult)
                    nc.vector.tensor_reduce(out=d1[:], in_=pr3, op=ALU.add,
                                            axis=AX.X)
                    prB = prp.tile([128, G * 64], BF16, tag="pr")
                    prB3 = prB[:].rearrange("p (g e) -> p g e", e=64)
                    dB = pp.tile([128, G], F32, tag="dB")
                    nc.vector.tensor_tensor(out=prB3, in0=ti3[:, :, 64:128],
                                            in1=ue3, op=ALU.mult)
                    nc.vector.tensor_reduce(out=dB[:], in_=prB3, op=ALU.add,
                                            axis=AX.X)
                    d2 = pp.tile([128, G], F32, tag="d2")
                    nc.vector.tensor_tensor(
                        out=d2[:], in0=dB[:],
                        in1=mk_all[:, bt * G:(bt + 1) * G], op=ALU.mult)
                    nc.vector.tensor_tensor(
                        out=d2[:], in0=d2[:],
                        in1=qa_all[:, bt * G:(bt + 1) * G], op=ALU.add)

                    s1 = pp.tile([128, G], F32, tag="s1")
                    s2 = pp.tile([128, G], F32, tag="s2")
                    tot = pp.tile([128, 2], F32, tag="tot")
                    nc.scalar.activation(out=s1[:], in_=d1[:], func=AF.Exp,
                                         scale=1.0 / TEMP,
                                         accum_out=tot[:, 0:1])
                    nc.scalar.activation(out=s2[:], in_=d2[:], func=AF.Exp,
                                         scale=1.0 / TEMP,
                                         accum_out=tot[:, 1:2])
                    nc.scalar.activation(out=acc[:, 4 * bt:4 * bt + 2],
                                         in_=tot[:], func=AF.Ln)
                    nc.scalar.activation(out=acc[:, 4 * bt + 2:4 * bt + 3],
                                         in_=d1[:, 0:1],
                                         func=AF.Copy, scale=-1.0 / TEMP)
                    nc.scalar.activation(out=acc[:, 4 * bt + 3:4 * bt + 4],
                                         in_=d2[:, 0:1],
                                         func=AF.Copy, scale=-1.0 / TEMP)

                LAST = NBP - 1
                fire = {}
                fire_ue = {}
                ue_tiles = {}
                for bp in range(NBP - 1):
                    ge = min(-(-(S_sup[bp] // CH) // GRP) * GRP, NCH)
                    fire.setdefault(ge, []).append(bp)
                    fire_ue.setdefault(max(ge - GRP, 0), []).append(bp)
                for bp in fire_ue.get(0, []):
                    ue_tiles[bp] = ph2_load_ue(bp)
                tfire = {}
                for t in range(4):
                    lo = (4 * t * EPT) // (4 * GCALL)
                    hi = ((t + 1) * EPT - 1) // GCALL
                    pref = max(S_call[LAST * CPS + k]
                               for k in range(lo, hi + 1))
                    ge = -(-(pref // CH) // GRP) * GRP
                    tfire.setdefault(min(ge, NCH), []).append(t)
                ti_last = None
                ue_last = None
                pend = []
                pend_t = []
                for g0 in range(0, NCH, GRP):
                    g1 = min(g0 + GRP, NCH)
                    hs = enc_groupA(g0)
                    while pend:
                        bp, ti_p, ue_p = pend.pop(0)
                        for t in range(4):
                            ph2_compute(bp, t, ti_p, ue_p)
                    while pend_t:
                        t = pend_t.pop(0)
                        ph2_compute(LAST, t, ti_last, ue_last)
                    enc_groupB(g0, hs)
                    for bp in fire_ue.get(g1, []):
                        if bp not in ue_tiles:
                            ue_tiles[bp] = ph2_load_ue(bp)
                    for bp in fire.get(g1, []):
                        pend.append((bp, ph2_gather(bp), ue_tiles[bp]))
                    for t in tfire.get(g1, []):
                        if ti_last is None:
                            ti_last = pp.tile([128, 4 * G * 128], BF16,
                                              tag="ti", bufs=3)
                            ue_last = pp.tile([128, 4 * G * 64], BF16,
                                              tag="ue", bufs=3)
                            nc.sync.dma_start(
                                out=ue_last[:],
                                in_=ue_d[:, 4 * LAST * G * 64:
                                         (4 * LAST + 4) * G * 64])
                        ph2_gather_tile(LAST, t, ti_last)
                        pend_t.append(t)
                while pend:
                    bp, ti_p, ue_p = pend.pop(0)
                    for t in range(4):
                        ph2_compute(bp, t, ti_p, ue_p)
                while pend_t:
                    t = pend_t.pop(0)
                    ph2_compute(LAST, t, ti_last, ue_last)

            nc.sync.dma_start(out=out_d[:], in_=acc[:])

    nc.compile()
    return nc


def _wrap_idx(idx):
    idx = np.asarray(idx, np.int16)
    n = len(idx)
    cols = n // 16
    w = np.ascontiguousarray(idx.reshape(cols, 16).T)
    return np.tile(w, (8, 1))


def _host_prep(v_feat, id_embedding, user_tensor, item_tensor, rand_index,
               b2h):
    it = np.clip(item_tensor.astype(np.int64) - NUM_USER, 0, NUM_ITEM - 1)
    itg = item_tensor.astype(np.int64)
    ut = user_tensor.astype(np.int64)
    mask = np.zeros(B * G, np.float32)
    mask[np.asarray(rand_index, dtype=np.int64)] = 1.0
    mask = mask.reshape(B, G)

    nrm = np.sqrt(np.sum(v_feat.astype(np.float64) ** 2, axis=1, keepdims=True))
    vhat = (v_feat / np.maximum(nrm, 1e-12)).astype(ml_dtypes.bfloat16)

    cores = []
    for k in range(NCORE):
        sl = slice(k * BC, (k + 1) * BC)
        itc, utc, itgc, mkc = it[sl], ut[sl], itg[sl], mask[sl]
        ui = np.unique(itc)
        je = np.searchsorted(ui, itc)
        fb = np.zeros(len(ui), np.int64)
        for bt in range(NT - 1, -1, -1):
            fb[je[bt * 128:(bt + 1) * 128].ravel()] = bt
        perm = np.argsort(fb, kind="stable")
        news = np.empty(len(ui), np.int64)
        news[perm] = np.arange(len(ui))
        je = news[je]
        uin = ui[perm]
        fb_sorted = fb[perm]
        S_t = [int(np.searchsorted(fb_sorted, t, side="right"))
               for t in range(NT)]
        cores.append((itc, utc, itgc, mkc, uin, je, S_t))

    NS0 = max(len(c[4]) for c in cores)
    NS4 = -(-NS0 // (8 * CH)) * (8 * CH)
    assert NS4 <= 32768
    S_call = [CH] * (NBP * CPS)
    for c in cores:
        je = c[5]
        l2 = je.reshape(NT, 128, G).transpose(0, 2, 1).reshape(NT, EPT)
        for bp in range(NBP):
            flat = l2[4 * bp:4 * bp + 4].reshape(-1)
            for k in range(CPS):
                blk = flat[k * GCALL:(k + 1) * GCALL]
                pref = -(-(int(blk.max()) + 1) // CH) * CH
                j = bp * CPS + k
                S_call[j] = min(max(S_call[j], pref), NS4)

    per_core = []
    for (itc, utc, itgc, mkc, uin, je, _S) in cores:
        vet = np.zeros((128, NS4), ml_dtypes.bfloat16)
        vet[:, 0:len(uin)] = vhat[uin].T
        l2 = je.reshape(NT, 128, G).transpose(0, 2, 1).reshape(NT, EPT)
        i2i = np.zeros((16, NT * ICOL), np.int16)
        for bt in range(NT):
            i2i[:, bt * ICOL:(bt + 1) * ICOL] = l2[bt].reshape(ICOL, 16).T
        i2i = np.tile(i2i, (8, 1))

        uemb = id_embedding[utc]
        ue = np.ascontiguousarray(
            uemb.astype(ml_dtypes.bfloat16).reshape(NT, 128, G, 64)
            .transpose(1, 0, 2, 3))
        qa = ((uemb.astype(np.float64)
               * id_embedding[itgc].astype(np.float64)).sum(-1) * (1.0 - mkc)
              + (uemb.astype(np.float64) @ b2h) * mkc)
        qab = np.ascontiguousarray(
            qa.astype(np.float32).reshape(NT, 128, G).transpose(1, 0, 2))
        peb = np.ascontiguousarray(
            id_embedding[itgc[:, 0]].reshape(NT, 128, 64).transpose(1, 0, 2))
        mkb = np.ascontiguousarray(
            mkc.reshape(NT, 128, G).transpose(1, 0, 2))

        per_core.append({
            "vet": vet, "i2i": i2i,
            "ue": ue.reshape(128, NT * G * 64),
            "qa": qab.reshape(128, NT * G),
            "peb": peb.reshape(128, NT * DIM_E),
            "mask": mkb.reshape(128, NT * G),
        })
    return NS4, S_call, per_core


def kernel(v_feat, id_embedding, W1, b1, W2, b2, user_tensor, item_tensor,
           rand_index):
    v_feat = np.asarray(v_feat, dtype=np.float32)
    id_embedding = np.asarray(id_embedding, dtype=np.float32)
    W1b = np.ascontiguousarray(W1, dtype=np.float32).astype(ml_dtypes.bfloat16)
    b1f = np.ascontiguousarray(b1, dtype=np.float32)
    W2f = np.ascontiguousarray(W2, dtype=np.float32)
    W2b = np.concatenate([W2f[0:128, :], W2f[128:256, :]], axis=1) \
        .astype(ml_dtypes.bfloat16)
    b2f = np.ascontiguousarray(b2, dtype=np.float32)

    NS4, S_call, per_core = _host_prep(v_feat, id_embedding, user_tensor,
                                       item_tensor, rand_index,
                                       b2f.astype(np.float64))

    key = (NS4, tuple(S_call))
    if _CACHE.get("key") != key:
        _CACHE["nc"] = _build(NS4, S_call)
        _CACHE["key"] = key
    nc = _CACHE["nc"]

    in_maps = []
    for k in range(NCORE):
        m = {"w1": W1b, "b1": b1f, "w2": W2b,
             "b2r": np.tile(b2f.astype(ml_dtypes.bfloat16)[None, :],
                            (128, 1))}
        m.update(per_core[k])
        in_maps.append(m)
    trace = bool(int(os.environ.get("KERNEL_TRACE", "0")))
    res = bass_utils.run_bass_kernel_spmd(
        nc, in_maps, core_ids=list(range(NCORE)), trace=trace)
    _CACHE["last_results"] = res
    accs = np.stack([r["acc_out"] for r in res.results])
    sums = accs.reshape(NCORE, 128, NT, 4).sum(axis=(0, 1, 2), dtype=np.float64)
    l1 = (sums[0] + sums[2]) / B
    l2 = (sums[1] + sums[3]) / B
    return np.array(LR_LAMBDA * l1 + (1.0 - LR_LAMBDA) * l2, dtype=np.float32)


# revision 56
# speedup vs baseline: 1.0333x; 1.0333x over previous
import os
import sys

import numpy as np

for _p in ("/opt/trn_rl_repo", os.path.expanduser("~/.axon_site/_ro/trn_rl_repo")):
    if os.path.isdir(_p) and _p not in sys.path:
        sys.path.insert(0, _p)

import ml_dtypes

import concourse.bacc as bacc
import concourse.mybir as mybir
import concourse.tile as tile
from concourse import bass_utils
from concourse.library_config import mlp as mlp_lib


F32 = mybir.dt.float32
BF16 = mybir.dt.bfloat16
I16 = mybir.dt.int16
AF = mybir.ActivationFunctionType
ALU = mybir.AluOpType
AX = mybir.AxisListType

NUM_USER = 200000
NUM_ITEM = 200000
DIM_E = 64
DIM_FEAT = 128
B = 16384
G = 17
TEMP = 0.2
LR_LAMBDA = 0.5

NCORE = 8
BC = B // NCORE
NT = BC // 128
NBP = NT // 4
EPT = 128 * G
ICOL = EPT // 16
CH = 512

GCALL = int(os.environ.get("KERNEL_GCALL", "1024"))
assert GCALL % 128 == 0
CPS = -(-4 * EPT // GCALL)

_CACHE: dict = {}

_NEEDED_AF = None


def _patch_act_tables():
    global _NEEDED_AF
    if _CACHE.get("act_patched"):
        return
    _NEEDED_AF = {AF.Ln, AF.Exp, AF.Prelu, AF.Copy, AF.Identity}
    import concourse.hw_specs as hw_specs
    orig = hw_specs.get_activation_tables

    def patched(module_arch):
        tabs = orig(module_arch)
        out = {}
        for name, fns in tabs.items():
            if name == "natural_log_exp_and_others":
                out[name] = fns
            else:
                out[name] = fns - _NEEDED_AF
        return out

    bacc.get_activation_tables = patched
    _CACHE["act_patched"] = True


def _build(NS4, S_call):
    _patch_act_tables()
    NCH = NS4 // CH
    GRP = int(os.environ.get('KERNEL_GRP', '4'))
    GW = GRP * CH

    nc = bacc.Bacc("TRN2", target_bir_lowering=False, debug=False,
                   num_devices=NCORE, num_swdge_queues=4)

    vet_d = nc.dram_tensor("vet", [128, NS4], BF16, kind="ExternalInput")
    w1_d = nc.dram_tensor("w1", [DIM_FEAT, 256], BF16, kind="ExternalInput")
    b1_d = nc.dram_tensor("b1", [256], F32, kind="ExternalInput")
    w2_d = nc.dram_tensor("w2", [128, 128], BF16, kind="ExternalInput")
    b2r_d = nc.dram_tensor("b2r", [128, DIM_E], BF16, kind="ExternalInput")
    i2i_d = nc.dram_tensor("i2i", [128, NT * ICOL], I16, kind="ExternalInput")
    ue_d = nc.dram_tensor("ue", [128, NT * G * 64], BF16, kind="ExternalInput")
    qa_d = nc.dram_tensor("qa", [128, NT * G], F32, kind="ExternalInput")
    pe_d = nc.dram_tensor("peb", [128, NT * DIM_E], F32, kind="ExternalInput")
    mk_d = nc.dram_tensor("mask", [128, NT * G], F32, kind="ExternalInput")
    out_d = nc.dram_tensor("acc_out", [128, 4 * NT], F32, kind="ExternalOutput")

    scri_d = nc.dram_tensor("scri", [NS4, 128], BF16, kind="Internal")

    with tile.TileContext(nc) as tc:
        nc.gpsimd.load_library(mlp_lib)
        with tc.tile_pool(name="const", bufs=1) as cp:
            w1sb = cp.tile([128, 256], BF16, tag="w1sb")
            nc.sync.dma_start(out=w1sb[:], in_=w1_d[:])
            w2sb = cp.tile([128, 128], BF16, tag="w2sb")
            nc.sync.dma_start(out=w2sb[:], in_=w2_d[:])
            b1sb = cp.tile([128, 2], F32, tag="b1sb")
            nc.sync.dma_start(out=b1sb[:], in_=b1_d[:].rearrange("(h p) -> p h", p=128))
            b2r = cp.tile([128, DIM_E], BF16, tag="b2r")
            nc.sync.dma_start(out=b2r[:], in_=b2r_d[:])
            acc = cp.tile([128, 4 * NT], F32, tag="acc")

            qrr = [0]

            def gather_split(dst3, src, idxs, coff0, total):
                done = 0
                while done < total:
                    n = min(GCALL, total - done)
                    nc.gpsimd.dma_gather(
                        dst3[:, done // 128:(done + n) // 128, :],
                        src,
                        idxs[:, coff0 + done // 16:coff0 + (done + n) // 16],
                        n, n, 128, queue_num=qrr[0] % 4)
                    qrr[0] += 1
                    done += n

            with tc.tile_pool(name="enc", bufs=3) as ep, \
                 tc.tile_pool(name="ph2", bufs=3) as pp, \
                 tc.tile_pool(name="prd", bufs=2) as prp, \
                 tc.tile_pool(name="psH", bufs=3, space="PSUM") as psH, \
                 tc.tile_pool(name="psO", bufs=3, space="PSUM") as psO:

                i2i = pp.tile([128, NT * ICOL], I16, tag="i2i", bufs=1)
                nc.sync.dma_start(out=i2i[:], in_=i2i_d[:])
                ssf = pp.tile([128, NS4 // 128], F32, tag="ssf", bufs=1)

                pe_all = pp.tile([128, NT * DIM_E], F32, tag="peA", bufs=1)
                nc.sync.dma_start(out=pe_all[:], in_=pe_d[:])
                mk_all = pp.tile([128, NT * G], F32, tag="mkA", bufs=1)
                nc.sync.dma_start(out=mk_all[:], in_=mk_d[:])
                qa_all = pp.tile([128, NT * G], F32, tag="qaA", bufs=1)
                nc.sync.dma_start(out=qa_all[:], in_=qa_d[:])
                prP = pp.tile([128, NT * DIM_E], F32, tag="prP", bufs=1)
                nc.vector.tensor_tensor(out=prP[:], in0=pe_all[:],
                                        in1=pe_all[:], op=ALU.mult)
                dPPa = pp.tile([128, NT], F32, tag="dPPa", bufs=1)
                nc.vector.tensor_reduce(
                    out=dPPa[:],
                    in_=prP[:].rearrange("p (t e) -> p t e", e=DIM_E),
                    op=ALU.add, axis=AX.X)
                nc.scalar.activation(out=dPPa[:], in_=dPPa[:], func=AF.Ln)
                nc.scalar.activation(out=dPPa[:], in_=dPPa[:], func=AF.Exp,
                                     scale=-0.5)
                phA = pp.tile([128, NT * DIM_E], BF16, tag="phA", bufs=1)
                nc.vector.tensor_tensor(
                    out=phA[:].rearrange("p (t e) -> p t e", e=DIM_E),
                    in0=pe_all[:].rearrange("p (t e) -> p t e", e=DIM_E),
                    in1=dPPa[:].rearrange("p (t o) -> p t o", o=1)
                        .to_broadcast([128, NT, DIM_E]),
                    op=ALU.mult)

                def enc_groupA(g0):
                    veT = ep.tile([128, GW], BF16, tag="veT", bufs=2)
                    nc.sync.dma_start(out=veT[:],
                                      in_=vet_d[:, g0 * CH:g0 * CH + GW])
                    hs = []
                    for c in range(GRP):
                        h_sb = ep.tile([128, 2 * CH], BF16, tag="hsb", bufs=10)
                        for h in range(2):
                            h_ps = psH.tile([128, CH], F32, tag="hp")
                            nc.tensor.matmul(
                                out=h_ps[:],
                                lhsT=w1sb[:, h * 128:(h + 1) * 128],
                                rhs=veT[:, c * CH:(c + 1) * CH],
                                start=True, stop=True)
                            nc.scalar.activation(
                                out=h_sb[:, h * CH:(h + 1) * CH], in_=h_ps[:],
                                func=AF.Prelu, bias=b1sb[:, h:h + 1],
                                scale=1.0, alpha=0.01)
                        hs.append(h_sb)
                    return hs

                def enc_groupB(g0, hs):
                    scG = ep.tile([128, GW], BF16, tag="scG", bufs=2)
                    scG3 = scG[:].rearrange("p (j e) -> p j e", e=128)
                    fPb = ep.tile([128, GW // 2], BF16, tag="fPb", bufs=2)
                    fPb3 = fPb[:].rearrange("p (j e) -> p j e", e=64)
                    for c in range(GRP):
                        h_sb = hs[c]
                        Fo_ps = psO.tile([128, CH // 2], F32, tag="fo")
                        for j in range(4):
                            nc.tensor.matmul(
                                out=Fo_ps[:, j * 64:(j + 1) * 64],
                                lhsT=h_sb[:, j * 128:(j + 1) * 128],
                                rhs=w2sb[:, 0:64], start=True, stop=False)
                            nc.tensor.matmul(
                                out=Fo_ps[:, j * 64:(j + 1) * 64],
                                lhsT=h_sb[:, CH + j * 128:CH + (j + 1) * 128],
                                rhs=w2sb[:, 64:128], start=False, stop=True)
                        nc.scalar.activation(
                            out=fPb[:, c * 256:(c + 1) * 256], in_=Fo_ps[:],
                            func=AF.Identity)
                    so = g0 * 4
                    fB = ep.tile([128, GW // 2], BF16, tag="fB", bufs=2)
                    fB3 = fB[:].rearrange("p (j e) -> p j e", e=64)
                    nc.vector.tensor_tensor(
                        out=fB3, in0=fPb3,
                        in1=b2r[:].rearrange("p (j e) -> p j e", j=1)
                            .to_broadcast([128, 4 * GRP, 64]),
                        op=ALU.add)
                    sq = ep.tile([128, GW // 2], BF16, tag="sq", bufs=2)
                    sq3 = sq[:].rearrange("p (j e) -> p j e", e=64)
                    nc.vector.tensor_tensor(out=sq3, in0=fB3, in1=fB3,
                                            op=ALU.mult)
                    nc.vector.tensor_reduce(out=ssf[:, so:so + 4 * GRP],
                                            in_=sq3, op=ALU.add, axis=AX.X)
                    nc.scalar.activation(out=ssf[:, so:so + 4 * GRP],
                                         in_=ssf[:, so:so + 4 * GRP],
                                         func=AF.Ln)
                    nc.scalar.activation(out=ssf[:, so:so + 4 * GRP],
                                         in_=ssf[:, so:so + 4 * GRP],
                                         func=AF.Exp, scale=-0.5)
                    invb = ep.tile([128, 4 * GRP], BF16, tag="invb", bufs=2)
                    nc.vector.tensor_copy(out=invb[:], in_=ssf[:, so:so + 4 * GRP])
                    fhG = ep.tile([128, GW // 2], BF16, tag="fhG", bufs=2)
                    fhG3 = fhG[:].rearrange("p (j e) -> p j e", e=64)
                    nc.vector.tensor_tensor(
                        out=fhG3, in0=fB3,
                        in1=invb[:].rearrange("p (j o) -> p j o", o=1)
                            .to_broadcast([128, 4 * GRP, 64]),
                        op=ALU.mult)
                    nc.vector.tensor_copy(out=scG3[:, :, 0:64], in_=fhG3)
                    nc.vector.tensor_copy(out=scG3[:, :, 64:128], in_=fPb3)
                    nc.sync.dma_start(
                        out=scri_d[g0 * CH:g0 * CH + GW, :]
                            .rearrange("(j p) e -> p j e", p=128),
                        in_=scG)

                S_sup = [max(S_call[bp * CPS:(bp + 1) * CPS])
                         for bp in range(NBP)]

                def ph2_gather(bp):
                    ti_p = pp.tile([128, 4 * G * 128], BF16, tag="ti", bufs=3)
                    ti_p3 = ti_p[:].rearrange("p (g e) -> p g e", e=128)
                    gather_split(ti_p3, scri_d[0:S_sup[bp], :], i2i,
                                 4 * bp * ICOL, 4 * EPT)
                    ue_p = pp.tile([128, 4 * G * 64], BF16, tag="ue", bufs=3)
                    nc.sync.dma_start(
                        out=ue_p[:],
                        in_=ue_d[:, 4 * bp * G * 64:(4 * bp + 4) * G * 64])
                    return ti_p, ue_p

                def ph2_gather_tile(bp, t, ti_p):
                    lo = (4 * t * EPT) // (4 * GCALL)
                    hi = ((t + 1) * EPT - 1) // GCALL
                    pref = max(S_call[bp * CPS + k]
                               for k in range(lo, hi + 1))
                    ti_p3 = ti_p[:].rearrange("p (g e) -> p g e", e=128)
                    done = t * EPT
                    end = (t + 1) * EPT
                    while done < end:
                        n = min(GCALL, end - done)
                        nc.gpsimd.dma_gather(
                            ti_p3[:, done // 128:(done + n) // 128, :],
                            scri_d[0:pref, :],
                            i2i[:, 4 * bp * ICOL + done // 16:
                                4 * bp * ICOL + (done + n) // 16],
                            n, n, 128, queue_num=qrr[0] % 4)
                        qrr[0] += 1
                        done += n

                def ph2_compute(bp, t, ti_p, ue_p):
                    bt = 4 * bp + t
                    ti3 = ti_p[:, t * G * 128:(t + 1) * G * 128] \
                        .rearrange("p (g e) -> p g e", e=128)
                    ue3 = ue_p[:, t * G * 64:(t + 1) * G * 64] \
                        .rearrange("p (g e) -> p g e", e=64)
                    ph3b = phA[:, bt * DIM_E:(bt + 1) * DIM_E] \
                        .rearrange("p (g e) -> p g e", g=1) \
                        .to_broadcast([128, G, 64])
                    pr = prp.tile([128, G * 64], BF16, tag="pr")
                    pr3 = pr[:].rearrange("p (g e) -> p g e", e=64)
                    d1 = pp.tile([128, G], F32, tag="d1")
                    nc.vector.tensor_tensor(out=pr3, in0=ti3[:, :, 0:64],
                                            in1=ph3b, op=ALU.mult)
                    nc.vector.tensor_reduce(out=d1[:], in_=pr3, op=ALU.add,
                                            axis=AX.X)
                    prB = prp.tile([128, G * 64], BF16, tag="pr")
                    prB3 = prB[:].rearrange("p (g e) -> p g e", e=64)
                    dB = pp.tile([128, G], F32, tag="dB")
                    nc.vector.tensor_tensor(out=prB3, in0=ti3[:, :, 64:128],
                                            in1=ue3, op=ALU.mult)
                    nc.vector.tensor_reduce(out=dB[:], in_=prB3, op=ALU.add,
                                            axis=AX.X)
                    d2 = pp.tile([128, G], F32, tag="d2")
                    nc.vector.tensor_tensor(
                        out=d2[:], in0=dB[:],
                        in1=mk_all[:, bt * G:(bt + 1) * G], op=ALU.mult)
                    nc.vector.tensor_tensor(
                        out=d2[:], in0=d2[:],
                        in1=qa_all[:, bt * G:(bt + 1) * G], op=ALU.add)

                    s1 = pp.tile([128, G], F32, tag="s1")
                    s2 = pp.tile([128, G], F32, tag="s2")
                    tot = pp.tile([128, 2], F32, tag="tot")
                    nc.scalar.activation(out=s1[:], in_=d1[:], func=AF.Exp,
                                         scale=1.0 / TEMP,
                                         accum_out=tot[:, 0:1])
                    nc.scalar.activation(out=s2[:], in_=d2[:], func=AF.Exp,
                                         scale=1.0 / TEMP,
                                         accum_out=tot[:, 1:2])
                    nc.scalar.activation(out=acc[:, 4 * bt:4 * bt + 2],
                                         in_=tot[:], func=AF.Ln)
                    nc.scalar.activation(out=acc[:, 4 * bt + 2:4 * bt + 3],
                                         in_=d1[:, 0:1],
                                         func=AF.Copy, scale=-1.0 / TEMP)
                    nc.scalar.activation(out=acc[:, 4 * bt + 3:4 * bt + 4],
                                         in_=d2[:, 0:1],
                                         func=AF.Copy, scale=-1.0 / TEMP)

                LAST = NBP - 1
                fire = {}
                for bp in range(NBP - 1):
                    ge = -(-(S_sup[bp] // CH) // GRP) * GRP
                    fire.setdefault(min(ge, NCH), []).append(bp)
                tfire = {}
                for t in range(4):
                    lo = (4 * t * EPT) // (4 * GCALL)
                    hi = ((t + 1) * EPT - 1) // GCALL
                    pref = max(S_call[LAST * CPS + k]
                               for k in range(lo, hi + 1))
                    ge = -(-(pref // CH) // GRP) * GRP
                    tfire.setdefault(min(ge, NCH), []).append(t)
                ti_last = None
                ue_last = None
                pend = []
                pend_t = []
                for g0 in range(0, NCH, GRP):
                    g1 = min(g0 + GRP, NCH)
                    hs = enc_groupA(g0)
                    while pend:
                        bp, ti_p, ue_p = pend.pop(0)
                        for t in range(4):
                            ph2_compute(bp, t, ti_p, ue_p)
                    while pend_t:
                        t = pend_t.pop(0)
                        ph2_compute(LAST, t, ti_last, ue_last)
                    enc_groupB(g0, hs)
                    for bp in fire.get(g1, []):
                        pend.append((bp,) + ph2_gather(bp))
                    for t in tfire.get(g1, []):
                        if ti_last is None:
                            ti_last = pp.tile([128, 4 * G * 128], BF16,
                                              tag="ti", bufs=3)
                            ue_last = pp.tile([128, 4 * G * 64], BF16,
                                              tag="ue", bufs=3)
                            nc.sync.dma_start(
                                out=ue_last[:],
                                in_=ue_d[:, 4 * LAST * G * 64:
                                         (4 * LAST + 4) * G * 64])
                        ph2_gather_tile(LAST, t, ti_last)
                        pend_t.append(t)
                while pend:
                    bp, ti_p, ue_p = pend.pop(0)
                    for t in range(4):
                        ph2_compute(bp, t, ti_p, ue_p)
                while pend_t:
                    t = pend_t.pop(0)
                    ph2_compute(LAST, t, ti_last, ue_last)

            nc.sync.dma_start(out=out_d[:], in_=acc[:])

    nc.compile()
    return nc


def _wrap_idx(idx):
    idx = np.asarray(idx, np.int16)
    n = len(idx)
    cols = n // 16
    w = np.ascontiguousarray(idx.reshape(cols, 16).T)
    return np.tile(w, (8, 1))


def _host_prep(v_feat, id_embedding, user_tensor, item_tensor, rand_index,
               b2h):
    it = np.clip(item_tensor.astype(np.int64) - NUM_USER, 0, NUM_ITEM - 1)
    itg = item_tensor.astype(np.int64)
    ut = user_tensor.astype(np.int64)
    mask = np.zeros(B * G, np.float32)
    mask[np.asarray(rand_index, dtype=np.int64)] = 1.0
    mask = mask.reshape(B, G)

    nrm = np.sqrt(np.sum(v_feat.astype(np.float64) ** 2, axis=1, keepdims=True))
    vhat = (v_feat / np.maximum(nrm, 1e-12)).astype(ml_dtypes.bfloat16)

    cores = []
    for k in range(NCORE):
        sl = slice(k * BC, (k + 1) * BC)
        itc, utc, itgc, mkc = it[sl], ut[sl], itg[sl], mask[sl]
        ui = np.unique(itc)
        je = np.searchsorted(ui, itc)
        fb = np.zeros(len(ui), np.int64)
        for bt in range(NT - 1, -1, -1):
            fb[je[bt * 128:(bt + 1) * 128].ravel()] = bt
        perm = np.argsort(fb, kind="stable")
        news = np.empty(len(ui), np.int64)
        news[perm] = np.arange(len(ui))
        je = news[je]
        uin = ui[perm]
        fb_sorted = fb[perm]
        S_t = [int(np.searchsorted(fb_sorted, t, side="right"))
               for t in range(NT)]
        cores.append((itc, utc, itgc, mkc, uin, je, S_t))

    NS0 = max(len(c[4]) for c in cores)
    NS4 = -(-NS0 // (8 * CH)) * (8 * CH)
    assert NS4 <= 32768
    S_call = [CH] * (NBP * CPS)
    for c in cores:
        je = c[5]
        l2 = je.reshape(NT, 128, G).transpose(0, 2, 1).reshape(NT, EPT)
        for bp in range(NBP):
            flat = l2[4 * bp:4 * bp + 4].reshape(-1)
            for k in range(CPS):
                blk = flat[k * GCALL:(k + 1) * GCALL]
                pref = -(-(int(blk.max()) + 1) // CH) * CH
                j = bp * CPS + k
                S_call[j] = min(max(S_call[j], pref), NS4)

    per_core = []
    for (itc, utc, itgc, mkc, uin, je, _S) in cores:
        vet = np.zeros((128, NS4), ml_dtypes.bfloat16)
        vet[:, 0:len(uin)] = vhat[uin].T
        l2 = je.reshape(NT, 128, G).transpose(0, 2, 1).reshape(NT, EPT)
        i2i = np.zeros((16, NT * ICOL), np.int16)
        for bt in range(NT):
            i2i[:, bt * ICOL:(bt + 1) * ICOL] = l2[bt].reshape(ICOL, 16).T
        i2i = np.tile(i2i, (8, 1))

        uemb = id_embedding[utc]
        ue = np.ascontiguousarray(
            uemb.astype(ml_dtypes.bfloat16).reshape(NT, 128, G, 64)
            .transpose(1, 0, 2, 3))
        qa = ((uemb.astype(np.float64)
               * id_embedding[itgc].astype(np.float64)).sum(-1) * (1.0 - mkc)
              + (uemb.astype(np.float64) @ b2h) * mkc)
        qab = np.ascontiguousarray(
            qa.astype(np.float32).reshape(NT, 128, G).transpose(1, 0, 2))
        peb = np.ascontiguousarray(
            id_embedding[itgc[:, 0]].reshape(NT, 128, 64).transpose(1, 0, 2))
        mkb = np.ascontiguousarray(
            mkc.reshape(NT, 128, G).transpose(1, 0, 2))

        per_core.append({
            "vet": vet, "i2i": i2i,
            "ue": ue.reshape(128, NT * G * 64),
            "qa": qab.reshape(128, NT * G),
            "peb": peb.reshape(128, NT * DIM_E),
            "mask": mkb.reshape(128, NT * G),
        })
    return NS4, S_call, per_core


def kernel(v_feat, id_embedding, W1, b1, W2, b2, user_tensor, item_tensor,
           rand_index):
    v_feat = np.asarray(v_feat, dtype=np.float32)
    id_embedding = np.asarray(id_embedding, dtype=np.float32)
    W1b = np.ascontiguousarray(W1, dtype=np.float32).astype(ml_dtypes.bfloat16)
    b1f = np.ascontiguousarray(b1, dtype=np.float32)
    W2f = np.ascontiguousarray(W2, dtype=np.float32)
    W2b = np.concatenate([W2f[0:128, :], W2f[128:256, :]], axis=1) \
        .astype(ml_dtypes.bfloat16)
    b2f = np.ascontiguousarray(b2, dtype=np.float32)

    NS4, S_call, per_core = _host_prep(v_feat, id_embedding, user_tensor,
                                       item_tensor, rand_index,
                                       b2f.astype(np.float64))

    key = (NS4, tuple(S_call))
    if _CACHE.get("key") != key:
        _CACHE["nc"] = _build(NS4, S_call)
        _CACHE["key"] = key
    nc = _CACHE["nc"]

    in_maps = []
    for k in range(NCORE):
        m = {"w1": W1b, "b1": b1f, "w2": W2b,
             "b2r": np.tile(b2f.astype(ml_dtypes.bfloat16)[None, :],
                            (128, 1))}
        m.update(per_core[k])
        in_maps.append(m)
    trace = bool(int(os.environ.get("KERNEL_TRACE", "0")))
    res = bass_utils.run_bass_kernel_spmd(
        nc, in_maps, core_ids=list(range(NCORE)), trace=trace)
    _CACHE["last_results"] = res
    accs = np.stack([r["acc_out"] for r in res.results])
    sums = accs.reshape(NCORE, 128, NT, 4).sum(axis=(0, 1, 2), dtype=np.float64)
    l1 = (sums[0] + sums[2]) / B
    l2 = (sums[1] + sums[3]) / B
    return np.array(LR_LAMBDA * l1 + (1.0 - LR_LAMBDA) * l2, dtype=np.float32)
